# revision 12
# baseline (speedup 1.0000x reference)
"""Trainium2 Bass kernel for nn_Correct_PrototypeManager (segment_reduce).

Reference computation:
    pred_lbl = argmax(preds, axis=1)                      # [B, H, W]
    feats_up = bilinear_resize(feats, H, W)               # [B, C, H, W]
    joint[b,k,h,w] = (masks==k) & (pred_lbl==k)
    counts[b,k] = sum_hw joint ; sums[b,k,c] = sum_hw feats_up * joint
    proto = mean_b( sums / (counts + eps) )               # [K, C]

Algebraic transform: bilinear upsample is linear (feats_up = (Uh (x) Uw)
@ feats), so sums[k,c] = <Uh^T joint_k Uw, feats_c> — we downsample the
one-hot joint map (256^2 -> 64^2) with the adjoint of the upsample and
contract over 4096 coarse pixels. counts are preserved exactly (rows of
U sum to 1); counts come out of the same final matmul via a ones column
appended to the feats operand.

Numerics: the downsample pipeline runs in fp16 EXACTLY — joint is 0/1,
bilinear adjoint weights are multiples of 1/8, so A (<=4, units of 1/8)
and B (<=16, units of 1/64) fit fp16's 11-bit mantissa. The argmax
compare stays fp32 (fp16 would create false ties). Only feats are
rounded to fp16 (~2.4e-4 relative).

Engine notes (why it looks like this): DVE is the joint-phase critical
engine; Pool supports only mult/add/sub (no is_equal/max) and its Q7
software loops are ~4x slower, so everything elementwise is on DVE with
big contiguous ops (2-byte packed innermost gets the 2x DVE mode; fp32
runs 1x). preds DMAs get their own queue (sync) so the first quarter
lands ~4us; all other loads ride the Pool DGE queue.

Sharding: data-parallel over batch B=8, one image per NeuronCore; the
[22, 258] per-image partial (sums[k,c] + counts col) is gathered on
host, divided and batch-meaned there (tiny).
"""

import numpy as np

B = 8
C = 256
K = 21
K2 = K + 1          # pad class dim (even free dims + contiguous b2 blocks)
HC = WC = 64
HF = WF = 256
EPS = 1e-6
N_CORES = 8
PIX = HC * WC       # 4096
KW = K * WF         # 5376 joint free elems per half
HK = HC * K2        # 1408 b2 free elems
FTW = C + 2         # ft chunk width: 256 feats + ones col + pad = 258

_PROGRAM_CACHE: dict = {}


def _upsample_matrix(n_in: int, n_out: int) -> np.ndarray:
    """U [n_out, n_in] with resize(x, 'bilinear', half-pixel) == U @ x."""
    U = np.zeros((n_out, n_in), dtype=np.float64)
    scale = n_in / n_out
    for i in range(n_out):
        src = (i + 0.5) * scale - 0.5
        f = int(np.floor(src))
        w = src - f
        lo = min(max(f, 0), n_in - 1)
        hi = min(max(f + 1, 0), n_in - 1)
        U[i, lo] += 1.0 - w
        U[i, hi] += w
    return U.astype(np.float32)


def _build_program(stage: int = 99):
    import concourse.bass as bass
    import concourse.bacc as bacc
    import concourse.tile as tile
    from concourse import mybir
    from contextlib import ExitStack

    f32 = mybir.dt.float32
    f16 = mybir.dt.float16

    nc = bacc.Bacc("TRN2", target_bir_lowering=False, debug=False,
                   num_devices=N_CORES)

    preds_d = nc.dram_tensor("preds", [K, HF, WF], f32, kind="ExternalInput")
    mask_d = nc.dram_tensor("mask", [2, 128, WF], f16, kind="ExternalInput")
    iota_d = nc.dram_tensor("iota", [128, KW], f16, kind="ExternalInput")
    ft_d = nc.dram_tensor("ft", [PIX // 128, 128, FTW], f16,
                          kind="ExternalInput")
    u16_d = nc.dram_tensor("u16", [HF, HC], f16, kind="ExternalInput")
    ident_d = nc.dram_tensor("ident", [64, 64], f16, kind="ExternalInput")
    out_d = nc.dram_tensor("out", [K2, FTW], f32, kind="ExternalOutput")

    with tile.TileContext(nc) as tc, ExitStack() as ctx:
        const_pool = ctx.enter_context(tc.tile_pool(name="const", bufs=1))
        joint_pool = ctx.enter_context(tc.tile_pool(name="joint", bufs=1))
        ft_pool = ctx.enter_context(tc.tile_pool(name="ft", bufs=1))
        res_pool = ctx.enter_context(tc.tile_pool(name="res", bufs=1))
        ps_pool = ctx.enter_context(
            tc.tile_pool(name="ps", bufs=5, space="PSUM"))
        pst_pool = ctx.enter_context(
            tc.tile_pool(name="pst", bufs=2, space="PSUM"))
        psf_pool = ctx.enter_context(
            tc.tile_pool(name="psf", bufs=1, space="PSUM"))

        # --- preds quarters: sole users of the sync DGE queue, issued
        # first so quarter 0 lands at ~4us ---
        with tc.tile_pool(name="trans", bufs=2) as tr_pool, \
                tc.tile_pool(name="mx", bufs=2) as mx_pool:
            preds_t = []
            for h in range(2):
                hs = h * 128
                pt = tr_pool.tile([128, KW], f32, tag=f"preds{h}")
                preds_t.append(pt)
                for b in range(2):
                    ws = b * 128
                    nc.sync.dma_start(
                        pt[:].rearrange("p (k w) -> p k w", k=K)[
                            :, :, ws:ws + 128],
                        preds_d.ap()[:, hs:hs + 128, ws:ws + 128].transpose(
                            [1, 0, 2]))

            # --- everything else on the Pool DGE queue: small mask/iota
            # first (feed the oh build), ft + consts afterwards ---
            mask_t = []
            for h in range(2):
                mt = const_pool.tile([128, WF], f16, tag=f"mask{h}")
                nc.gpsimd.dma_start(mt[:], mask_d.ap()[h, :, :])
                mask_t.append(mt)
            iota_t = const_pool.tile([128, KW], f16, tag="iota")
            nc.gpsimd.dma_start(iota_t[:], iota_d.ap()[:, :])
            u16_t = []
            for h in range(2):
                t16 = const_pool.tile([128, HC], f16, tag=f"u16_{h}")
                nc.gpsimd.dma_start(t16[:],
                                    u16_d.ap()[h * 128:(h + 1) * 128, :])
                u16_t.append(t16)
            ident_t = const_pool.tile([64, 64], f16, tag="ident")
            nc.gpsimd.dma_start(ident_t[:], ident_d.ap()[:, :])
            ft_big = ft_pool.tile([128, (PIX // 128) * FTW], f16, tag="ftbig")
            nc.gpsimd.dma_start(
                ft_big[:].rearrange("p (x n) -> p x n", x=PIX // 128),
                ft_d.ap().transpose([1, 0, 2]))

            # --- one-hot of mask (runs in the preds-DMA shadow) ---
            oh_t = []
            for h in range(2):
                oh = joint_pool.tile([128, KW], f16, tag=f"oh{h}")
                oh_t.append(oh)
                nc.vector.tensor_tensor(
                    oh[:].rearrange("p (k w) -> p k w", k=K),
                    mask_t[h][:].unsqueeze(1).to_broadcast([128, K, WF]),
                    iota_t[:].rearrange("p (k w) -> p k w", k=K),
                    op=mybir.AluOpType.is_equal)

            # --- per half: contiguous max tree, eq, joint-mul (all DVE) ---
            joint_t = []
            for h in range(2):
                p3 = preds_t[h][:].rearrange("p (k w) -> p k w", k=K)
                m1 = mx_pool.tile([128, 10 * WF], f32, tag="m1")
                m1v = m1[:].rearrange("p (k w) -> p k w", k=10)
                nc.vector.tensor_tensor(
                    m1v, p3[:, 0:10, :], p3[:, 10:20, :],
                    op=mybir.AluOpType.max)
                nc.vector.tensor_tensor(
                    m1v[:, 0:5, :], m1v[:, 0:5, :], m1v[:, 5:10, :],
                    op=mybir.AluOpType.max)
                nc.vector.tensor_tensor(
                    m1v[:, 0:2, :], m1v[:, 0:2, :], m1v[:, 2:4, :],
                    op=mybir.AluOpType.max)
                nc.vector.tensor_tensor(
                    m1v[:, 0:1, :], m1v[:, 0:1, :], m1v[:, 1:2, :],
                    op=mybir.AluOpType.max)
                nc.vector.tensor_tensor(
                    m1v[:, 0:1, :], m1v[:, 0:1, :], m1v[:, 4:5, :],
                    op=mybir.AluOpType.max)
                mxv = mx_pool.tile([128, WF], f32, tag="mxv")
                nc.vector.tensor_tensor(
                    mxv[:].unsqueeze(1), m1v[:, 0:1, :], p3[:, 20:21, :],
                    op=mybir.AluOpType.max)

                jt = joint_pool.tile([128, KW], f16, tag=f"joint{h}")
                joint_t.append(jt)
                nc.vector.tensor_tensor(
                    jt[:].rearrange("p (k w) -> p k w", k=K), p3,
                    mxv[:].unsqueeze(1).to_broadcast([128, K, WF]),
                    op=mybir.AluOpType.is_equal)
                nc.vector.tensor_tensor(
                    jt[:], jt[:], oh_t[h][:], op=mybir.AluOpType.mult)

        if stage <= 1:  # debug: dump joint slice for classes 0..K2-1
            dbg = res_pool.tile([128, K2], f32, tag="dbg")
            nc.scalar.copy(dbg[:], joint_t[0][:, 0:K2])
            nc.sync.dma_start(out_d.ap()[:, 0:128].transpose([1, 0]), dbg[:])

        # ----- stage 1: contract hf.  A[hc, (k, wf)] = Uh^T @ joint -----
        with tc.tile_pool(name="stg", bufs=1) as st_pool:
            a_t = st_pool.tile([64, KW], f16, tag="a")
            for fc in range(0, KW if stage >= 2 else 0, 512):
                w = min(512, KW - fc)
                ps = ps_pool.tile([64, 512], f32, tag="ps")
                nc.tensor.matmul(ps[:, :w], u16_t[0][:, :],
                                 joint_t[0][:, fc:fc + w],
                                 start=True, stop=False)
                nc.tensor.matmul(ps[:, :w], u16_t[1][:, :],
                                 joint_t[1][:, fc:fc + w],
                                 start=False, stop=True)
                nc.scalar.copy(a_t[:, fc:fc + w], ps[:, :w])

            # ----- stage 1.5: transpose A per class -> AT[wf, (wh, hc, k2)]
            # fp16 transposes into fp16 PSUM; scatter copies on DVE (idle
            # after the joint phase; handles the k-strided dst at ~1ns/elem)
            at_big = st_pool.tile([128, 2 * HK], f16, tag="at")
            if stage >= 3:
                # zero the k=21 pad column so stage 2 reads clean zeros
                nc.vector.memset(
                    at_big[:].rearrange(
                        "p (w h k) -> p w h k", w=2, h=HC)[:, :, :, K], 0.0)
            for k in range(K if stage >= 3 else 0):
                ps16 = pst_pool.tile([128, 128], f16, tag="ps16")
                for wh in range(2):
                    nc.tensor.transpose(
                        ps16[:, wh * 64:(wh + 1) * 64],
                        a_t[:, k * WF + wh * 128: k * WF + wh * 128 + 128],
                        ident_t[:])
                dst = at_big[:].rearrange(
                    "p (w h k) -> p w h k", w=2, h=HC)[:, :, :, k]
                nc.vector.tensor_copy(
                    dst, ps16[:].rearrange("p (w h) -> p w h", w=2))

            # ----- stage 2: contract wf.  B[wc, (hc, k2)] = Uw^T @ AT -----
            # b2 written twice: rows 64-127 hold B shifted by one hc so a
            # 128-pixel chunk (2 hc rows x 64 wc) is one full-partition
            # contiguous stationary slice for the final matmuls.
            b2 = st_pool.tile([128, HK], f16, tag="b2")
            for fc in range(0, HK if stage >= 3 else 0, 512):
                w = min(512, HK - fc)
                ps = ps_pool.tile([64, 512], f32, tag="ps")
                nc.tensor.matmul(ps[:, :w], u16_t[0][:, :],
                                 at_big[:, fc:fc + w],
                                 start=True, stop=False)
                nc.tensor.matmul(ps[:, :w], u16_t[1][:, :],
                                 at_big[:, HK + fc:HK + fc + w],
                                 start=False, stop=True)
                nc.scalar.copy(b2[0:64, fc:fc + w], ps[:, :w])
                if fc >= K2:
                    nc.scalar.copy(b2[64:128, fc - K2:fc + w - K2],
                                   ps[:, :w])
                else:
                    nc.scalar.copy(b2[64:128, 0:w - K2], ps[:, K2:w])

            # ----- final: out[k, c] = sum_ch b2_ch^T @ ft_ch, + counts ----
            ftv = ft_big[:].rearrange("p (x n) -> p x n", x=PIX // 128)
            if stage >= 4:
                psum_o = psf_pool.tile([K2, FTW], f32, tag="fin")
                for ch in range(PIX // 128):
                    nc.tensor.matmul(
                        psum_o[:, :],
                        b2[:, 2 * ch * K2: 2 * ch * K2 + K2],
                        ftv[:, ch, :],
                        start=(ch == 0), stop=(ch == PIX // 128 - 1))
                outc = res_pool.tile([K2, FTW], f32, tag="outc")
                nc.scalar.copy(outc[:], psum_o[:])
                nc.sync.dma_start(out_d.ap()[:, :], outc[:])

    nc.compile()
    return nc


def _get_program():
    if "nc" not in _PROGRAM_CACHE:
        _PROGRAM_CACHE["nc"] = _build_program()
    return _PROGRAM_CACHE["nc"]


def _host_inputs(feats, preds, masks):
    U = _upsample_matrix(HC, HF)
    u16 = U.astype(np.float16)
    ident = np.eye(64, dtype=np.float16)
    iota = np.broadcast_to(
        np.arange(K, dtype=np.float16)[None, :, None], (128, K, WF)
    ).reshape(128, KW).copy()

    feats = np.asarray(feats, dtype=np.float32)
    preds = np.asarray(preds, dtype=np.float32)
    masks_f = np.asarray(masks).astype(np.float16).reshape(B, 2, 128, WF)

    # feats^T [pix, c] fp16 with ones + zero-pad cols -> [32, 128, 258]
    ftp = np.empty((B, PIX, FTW), dtype=np.float16)
    ftp[:, :, :C] = feats.reshape(B, C, PIX).transpose(0, 2, 1)
    ftp[:, :, C] = 1.0
    ftp[:, :, C + 1] = 0.0

    in_maps = []
    for b in range(B):
        in_maps.append({
            "preds": np.ascontiguousarray(preds[b]),
            "mask": np.ascontiguousarray(masks_f[b]),
            "iota": iota,
            "ft": np.ascontiguousarray(ftp[b].reshape(PIX // 128, 128, FTW)),
            "u16": u16,
            "ident": ident,
        })
    return in_maps


def kernel(feats, preds, masks, _results_hook=None):
    from concourse.bass_utils import run_bass_kernel_spmd

    nc = _get_program()
    in_maps = _host_inputs(feats, preds, masks)
    res = run_bass_kernel_spmd(nc, in_maps, list(range(N_CORES)))
    if _results_hook is not None:
        _results_hook(res)

    protos = []
    for b in range(B):
        out = res.results[b]["out"]   # [K2, FTW] f32
        sums = out[:K, :C]            # [K, C]
        counts = out[:K, C]           # [K]
        protos.append(sums / (counts + EPS)[:, None])  # [K, C]
    return np.mean(np.stack(protos), axis=0).astype(np.float32)


# revision 16
# speedup vs baseline: 1.2040x; 1.2040x over previous
"""Trainium2 Bass kernel for nn_Correct_PrototypeManager (segment_reduce).

Reference computation:
    pred_lbl = argmax(preds, axis=1)                      # [B, H, W]
    feats_up = bilinear_resize(feats, H, W)               # [B, C, H, W]
    joint[b,k,h,w] = (masks==k) & (pred_lbl==k)
    counts[b,k] = sum_hw joint ; sums[b,k,c] = sum_hw feats_up * joint
    proto = mean_b( sums / (counts + eps) )               # [K, C]

Algebraic transform: bilinear upsample is linear (feats_up = (Uh (x) Uw)
@ feats), so sums[k,c] = <Uh^T joint_k Uw, feats_c> — we downsample the
one-hot joint map (256^2 -> 64^2) with the adjoint of the upsample and
contract over 4096 coarse pixels. counts are preserved exactly (rows of
U sum to 1) and come out of the same final matmul via a ones column
appended to the feats operand.

Numerics: the downsample pipeline runs in fp16 EXACTLY — joint is 0/1,
bilinear adjoint weights are multiples of 1/8, so A (<=4, units of 1/8)
and B (<=16, units of 1/64) fit fp16's 11-bit mantissa. The argmax
compare stays fp32 (fp16 would create false ties). Only feats are
rounded to fp16 (~2.4e-4 relative).

Schedule: the argmax/one-hot phase is ~41us of DVE work and is the
critical path (Pool has no is_equal/max and its Q7 loops are ~4x
slower; nothing else can compare tensors). So the whole downsample
pipeline is arranged to hide behind it: preds stream in wf-block-major
quarters, and as soon as both halves of wf-block 0 are joint-ed, the
PE runs stage1 (hf contraction) for that block, the per-class
transposes for that wf half, and the first half of the wf contraction
(stage2 accumulates across wf halves in held PSUM), all while the DVE
chews block 1. Transpose-scatter copies go to Pool during the joint
phase and DVE after it ends.

Sharding: data-parallel over batch B=8, one image per NeuronCore; the
[22, 258] per-image partial (sums[k,c] + counts col) is gathered on
host, divided and batch-meaned there (tiny).
"""

import numpy as np

B = 8
C = 256
K = 21
K2 = K + 1          # pad class dim (even free dims + contiguous b2 blocks)
HC = WC = 64
HF = WF = 256
EPS = 1e-6
N_CORES = 8
PIX = HC * WC       # 4096
KQ = K * 128        # 2688 elems per quarter tile
HK = HC * K2        # 1408 b2 free elems
FTW = C + 2         # ft chunk width: 256 feats + ones col + pad = 258

_PROGRAM_CACHE: dict = {}


def _upsample_matrix(n_in: int, n_out: int) -> np.ndarray:
    """U [n_out, n_in] with resize(x, 'bilinear', half-pixel) == U @ x."""
    U = np.zeros((n_out, n_in), dtype=np.float64)
    scale = n_in / n_out
    for i in range(n_out):
        src = (i + 0.5) * scale - 0.5
        f = int(np.floor(src))
        w = src - f
        lo = min(max(f, 0), n_in - 1)
        hi = min(max(f + 1, 0), n_in - 1)
        U[i, lo] += 1.0 - w
        U[i, hi] += w
    return U.astype(np.float32)


def _build_program(stage: int = 99):
    import concourse.bass as bass
    import concourse.bacc as bacc
    import concourse.tile as tile
    from concourse import mybir
    from contextlib import ExitStack

    f32 = mybir.dt.float32
    f16 = mybir.dt.float16
    AL = mybir.AluOpType

    nc = bacc.Bacc("TRN2", target_bir_lowering=False, debug=False,
                   num_devices=N_CORES)

    preds_d = nc.dram_tensor("preds", [K, HF, WF], f32, kind="ExternalInput")
    mask_d = nc.dram_tensor("mask", [2, 128, WF], f16, kind="ExternalInput")
    iota_d = nc.dram_tensor("iota", [128, KQ], f16, kind="ExternalInput")
    ft_d = nc.dram_tensor("ft", [PIX // 128, 128, FTW], f16,
                          kind="ExternalInput")
    u16_d = nc.dram_tensor("u16", [HF, HC], f16, kind="ExternalInput")
    ident_d = nc.dram_tensor("ident", [64, 64], f16, kind="ExternalInput")
    out_d = nc.dram_tensor("out", [K2, FTW], f32, kind="ExternalOutput")

    HB = [(0, 0), (1, 0), (0, 1), (1, 1)]   # (half, wf-block), block-major

    with tile.TileContext(nc) as tc, ExitStack() as ctx:
        const_pool = ctx.enter_context(tc.tile_pool(name="const", bufs=1))
        joint_pool = ctx.enter_context(tc.tile_pool(name="joint", bufs=1))
        ft_pool = ctx.enter_context(tc.tile_pool(name="ft", bufs=1))
        res_pool = ctx.enter_context(tc.tile_pool(name="res", bufs=1))
        stg_pool = ctx.enter_context(tc.tile_pool(name="stg", bufs=1))
        tr_pool = ctx.enter_context(tc.tile_pool(name="trans", bufs=1))
        mx_pool = ctx.enter_context(tc.tile_pool(name="mx", bufs=2))
        ps1_pool = ctx.enter_context(
            tc.tile_pool(name="ps1", bufs=2, space="PSUM"))
        ps2_pool = ctx.enter_context(
            tc.tile_pool(name="ps2", bufs=1, space="PSUM"))
        pst_pool = ctx.enter_context(
            tc.tile_pool(name="pst", bufs=2, space="PSUM"))
        psf_pool = ctx.enter_context(
            tc.tile_pool(name="psf", bufs=1, space="PSUM"))

        # --- preds quarters: sole users of the sync DGE queue, issued
        # first, block-major so wf-block 0 completes at half time ---
        preds_t = {}
        for h, b in HB:
            pt = tr_pool.tile([128, KQ], f32, tag=f"preds{h}{b}")
            preds_t[(h, b)] = pt
            nc.sync.dma_start(
                pt[:].rearrange("p (k w) -> p k w", k=K),
                preds_d.ap()[:, h * 128:(h + 1) * 128,
                             b * 128:(b + 1) * 128].transpose([1, 0, 2]))

        # --- small loads on the Pool DGE queue (mask/iota feed the oh
        # build; u16/ident feed the PE stages) ---
        mask_t = []
        for h in range(2):
            mt = const_pool.tile([128, WF], f16, tag=f"mask{h}")
            nc.gpsimd.dma_start(mt[:], mask_d.ap()[h, :, :])
            mask_t.append(mt)
        iota_t = const_pool.tile([128, KQ], f16, tag="iota")
        nc.gpsimd.dma_start(iota_t[:], iota_d.ap()[:, :])
        u16_t = []
        for h in range(2):
            t16 = const_pool.tile([128, HC], f16, tag=f"u16_{h}")
            nc.gpsimd.dma_start(t16[:], u16_d.ap()[h * 128:(h + 1) * 128, :])
            u16_t.append(t16)
        ident_t = const_pool.tile([64, 64], f16, tag="ident")
        nc.gpsimd.dma_start(ident_t[:], ident_d.ap()[:, :])

        # --- one-hot of mask per quarter (runs in the preds-DMA shadow) --
        iota3 = iota_t[:].rearrange("p (k w) -> p k w", k=K)
        oh_t = {}
        for h, b in HB:
            oh = joint_pool.tile([128, KQ], f16, tag=f"oh{h}{b}")
            oh_t[(h, b)] = oh
            nc.vector.tensor_tensor(
                oh[:].rearrange("p (k w) -> p k w", k=K),
                mask_t[h][:, b * 128:(b + 1) * 128].unsqueeze(1).to_broadcast(
                    [128, K, 128]),
                iota3, op=AL.is_equal)

        # stage tiles
        a_t = stg_pool.tile([64, K * WF], f16, tag="a")
        a3 = a_t[:].rearrange("p (k w) -> p k w", k=K)
        at_big = stg_pool.tile([128, 2 * HK], f16, tag="at")
        at4 = at_big[:].rearrange("p (w h k) -> p w h k", w=2, h=HC)
        b2 = stg_pool.tile([128, HK], f16, tag="b2")
        # zero the k=21 pad column of AT so stage 2 reads clean zeros
        nc.vector.memset(at4[:, :, :, K], 0.0)

        st2_ps = []
        if stage >= 3:
            for fc in range(0, HK, 512):
                ps = ps2_pool.tile([64, 512], f32, tag=f"st2_{fc}")
                st2_ps.append((fc, min(512, HK - fc), ps))

        # --- per quarter: max tree, eq, joint-mul (all DVE, the critical
        # chain); after both halves of a wf-block: stage1 + transposes +
        # stage2 half on PE, copies on Scalar/Pool ---
        joint_t = {}
        for h, b in HB:
            p3 = preds_t[(h, b)][:].rearrange("p (k w) -> p k w", k=K)
            m1 = mx_pool.tile([128, 10 * 128], f32, tag="m1")
            m1v = m1[:].rearrange("p (k w) -> p k w", k=10)
            nc.vector.tensor_tensor(
                m1v, p3[:, 0:10, :], p3[:, 10:20, :], op=AL.max)
            nc.vector.tensor_tensor(
                m1v[:, 0:5, :], m1v[:, 0:5, :], m1v[:, 5:10, :], op=AL.max)
            nc.vector.tensor_tensor(
                m1v[:, 0:2, :], m1v[:, 0:2, :], m1v[:, 2:4, :], op=AL.max)
            nc.vector.tensor_tensor(
                m1v[:, 0:1, :], m1v[:, 0:1, :], m1v[:, 1:2, :], op=AL.max)
            nc.vector.tensor_tensor(
                m1v[:, 0:1, :], m1v[:, 0:1, :], m1v[:, 4:5, :], op=AL.max)
            mxv = mx_pool.tile([128, 128], f32, tag="mxv")
            nc.vector.tensor_tensor(
                mxv[:].unsqueeze(1), m1v[:, 0:1, :], p3[:, 20:21, :],
                op=AL.max)

            jq = joint_pool.tile([128, KQ], f16, tag=f"joint{h}{b}")
            joint_t[(h, b)] = jq
            nc.vector.tensor_tensor(
                jq[:].rearrange("p (k w) -> p k w", k=K), p3,
                mxv[:].unsqueeze(1).to_broadcast([128, K, 128]),
                op=AL.is_equal)
            nc.vector.tensor_tensor(
                jq[:], jq[:], oh_t[(h, b)][:], op=AL.mult)

            if h != 1 or stage < 2:
                continue

            # ---- wf-block b complete in both halves: downsample it ----
            # stage 1: A[hc, kgroup, wf-block b] = Uh^T @ joint
            for g in range(6):
                k0, kw = 4 * g, min(4, K - 4 * g) * 128
                ps = ps1_pool.tile([64, 512], f32, tag="ps1")
                nc.tensor.matmul(
                    ps[:, :kw], u16_t[0][:, :],
                    joint_t[(0, b)][:, k0 * 128:k0 * 128 + kw],
                    start=True, stop=False)
                nc.tensor.matmul(
                    ps[:, :kw], u16_t[1][:, :],
                    joint_t[(1, b)][:, k0 * 128:k0 * 128 + kw],
                    start=False, stop=True)
                nc.scalar.copy(
                    a3[:, k0:k0 + kw // 128, b * 128:(b + 1) * 128],
                    ps[:, :kw].rearrange("p (k w) -> p k w", w=128))

            if stage < 3:
                continue
            # stage 1.5: transpose A's wf-half b per class; scatter copies
            # on Pool while the DVE is busy with block 1, on DVE after
            for k in range(K):
                pst = pst_pool.tile([128, 64], f16, tag="pst")
                nc.tensor.transpose(
                    pst[:], a_t[:, k * WF + b * 128: k * WF + b * 128 + 128],
                    ident_t[:])
                if b == 0:
                    nc.scalar.copy(at4[:, b, :, k], pst[:])
                else:
                    nc.vector.tensor_copy(at4[:, b, :, k], pst[:])

            # stage 2 half: accumulate Uw_b^T @ AT_b into held PSUM
            for fc, w, ps in st2_ps:
                nc.tensor.matmul(ps[:, :w], u16_t[b][:, :],
                                 at_big[:, b * HK + fc: b * HK + fc + w],
                                 start=(b == 0), stop=(b == 1))
                if b == 1:
                    nc.scalar.copy(b2[0:64, fc:fc + w], ps[:, :w])
                    if fc >= K2:
                        nc.scalar.copy(b2[64:128, fc - K2:fc + w - K2],
                                       ps[:, :w])
                    else:
                        nc.scalar.copy(b2[64:128, 0:w - K2], ps[:, K2:w])

        # feats^T load: deferred to here (the trigger sits on the sync
        # queue after the preds quarters; transfer overlaps the joint
        # phase tail, data needed only by the final matmuls)
        ft_big = ft_pool.tile([128, (PIX // 128) * FTW], f16, tag="ftbig")
        nc.sync.dma_start(
            ft_big[:].rearrange("p (x n) -> p x n", x=PIX // 128),
            ft_d.ap().transpose([1, 0, 2]))

        if stage <= 1:  # debug: dump joint slice for classes 0..K2-1
            dbg = res_pool.tile([128, K2], f32, tag="dbg")
            nc.scalar.copy(dbg[:], joint_t[(0, 0)][:, 0:K2])
            nc.sync.dma_start(out_d.ap()[:, 0:128].transpose([1, 0]), dbg[:])

        # ----- final: out[k, c] = sum_ch b2_ch^T @ ft_ch (+ counts) -----
        ftv = ft_big[:].rearrange("p (x n) -> p x n", x=PIX // 128)
        if stage >= 4:
            psum_o = psf_pool.tile([K2, FTW], f32, tag="fin")
            for ch in range(PIX // 128):
                nc.tensor.matmul(
                    psum_o[:, :],
                    b2[:, 2 * ch * K2: 2 * ch * K2 + K2],
                    ftv[:, ch, :],
                    start=(ch == 0), stop=(ch == PIX // 128 - 1))
            outc = res_pool.tile([K2, FTW], f32, tag="outc")
            nc.scalar.copy(outc[:], psum_o[:])
            nc.sync.dma_start(out_d.ap()[:, :], outc[:])

    nc.compile()
    return nc


def _get_program():
    if "nc" not in _PROGRAM_CACHE:
        _PROGRAM_CACHE["nc"] = _build_program()
    return _PROGRAM_CACHE["nc"]


def _host_inputs(feats, preds, masks):
    U = _upsample_matrix(HC, HF)
    u16 = U.astype(np.float16)
    ident = np.eye(64, dtype=np.float16)
    iota = np.broadcast_to(
        np.arange(K, dtype=np.float16)[None, :, None], (128, K, 128)
    ).reshape(128, KQ).copy()

    feats = np.asarray(feats, dtype=np.float32)
    preds = np.asarray(preds, dtype=np.float32)
    masks_f = np.asarray(masks).astype(np.float16).reshape(B, 2, 128, WF)

    # feats^T [pix, c] fp16 with ones + zero-pad cols -> [32, 128, 258]
    ftp = np.empty((B, PIX, FTW), dtype=np.float16)
    ftp[:, :, :C] = feats.reshape(B, C, PIX).transpose(0, 2, 1)
    ftp[:, :, C] = 1.0
    ftp[:, :, C + 1] = 0.0

    in_maps = []
    for b in range(B):
        in_maps.append({
            "preds": np.ascontiguousarray(preds[b]),
            "mask": np.ascontiguousarray(masks_f[b]),
            "iota": iota,
            "ft": np.ascontiguousarray(ftp[b].reshape(PIX // 128, 128, FTW)),
            "u16": u16,
            "ident": ident,
        })
    return in_maps


def kernel(feats, preds, masks, _results_hook=None):
    from concourse.bass_utils import run_bass_kernel_spmd

    nc = _get_program()
    in_maps = _host_inputs(feats, preds, masks)
    res = run_bass_kernel_spmd(nc, in_maps, list(range(N_CORES)))
    if _results_hook is not None:
        _results_hook(res)

    protos = []
    for b in range(B):
        out = res.results[b]["out"]   # [K2, FTW] f32
        sums = out[:K, :C]            # [K, C]
        counts = out[:K, C]           # [K]
        protos.append(sums / (counts + EPS)[:, None])  # [K, C]
    return np.mean(np.stack(protos), axis=0).astype(np.float32)


# revision 18
# speedup vs baseline: 1.2815x; 1.0643x over previous
"""Trainium2 Bass kernel for nn_Correct_PrototypeManager (segment_reduce).

Reference computation:
    pred_lbl = argmax(preds, axis=1)                      # [B, H, W]
    feats_up = bilinear_resize(feats, H, W)               # [B, C, H, W]
    joint[b,k,h,w] = (masks==k) & (pred_lbl==k)
    counts[b,k] = sum_hw joint ; sums[b,k,c] = sum_hw feats_up * joint
    proto = mean_b( sums / (counts + eps) )               # [K, C]

Algebraic transform: bilinear upsample is linear (feats_up = (Uh (x) Uw)
@ feats), so sums[k,c] = <Uh^T joint_k Uw, feats_c> — the one-hot joint
map is downsampled (256^2 -> 64^2) with the adjoint of the upsample and
contracted over 4096 coarse pixels. counts are preserved exactly (rows
of U sum to 1) and come out of the same final matmul via a ones column
appended to the feats operand.

Numerics: the downsample pipeline runs in fp16 EXACTLY — joint is 0/1,
bilinear adjoint weights are multiples of 1/8, so A (<=4, units of 1/8)
and B (<=16, units of 1/64) fit fp16's 11-bit mantissa. The argmax
compare stays fp32 (fp16 would create false ties). Only feats are
rounded to fp16 (~2.4e-4 relative).

Schedule: the argmax compare is ~31us of irreducible DVE work (Pool has
no is_equal/max; nothing else can compare tensors), so everything else
hides behind it. preds stream in wf-block-major quarters; per quarter
the DVE runs the max tree + equality; the joint multiply (eq * one-hot,
with the one-hot shipped from host) runs on Pool one quarter behind
(Pool's Q7 loops are slow but off the critical path; a warm-up op pays
its cold-start early). When both halves of a wf-block are done, the PE
runs stage1 (hf contraction), the per-class transposes of that wf half,
and one half of the wf contraction (stage2 accumulates across wf halves
in held PSUM) — all while the DVE chews the next block. Transpose
scatter-copies go to Scalar during the joint phase and DVE after it.

Sharding: data-parallel over batch B=8, one image per NeuronCore; the
[22, 258] per-image partial (sums[k,c] + counts col) is gathered on
host, divided and batch-meaned there (tiny).
"""

import numpy as np

B = 8
C = 256
K = 21
K2 = K + 1          # pad class dim (even free dims + contiguous b2 blocks)
HC = WC = 64
HF = WF = 256
EPS = 1e-6
N_CORES = 8
PIX = HC * WC       # 4096
KQ = K * 128        # 2688 elems per quarter tile
HK = HC * K2        # 1408 b2 free elems
FTW = C + 2         # ft chunk width: 256 feats + ones col + pad = 258

_PROGRAM_CACHE: dict = {}


def _upsample_matrix(n_in: int, n_out: int) -> np.ndarray:
    """U [n_out, n_in] with resize(x, 'bilinear', half-pixel) == U @ x."""
    U = np.zeros((n_out, n_in), dtype=np.float64)
    scale = n_in / n_out
    for i in range(n_out):
        src = (i + 0.5) * scale - 0.5
        f = int(np.floor(src))
        w = src - f
        lo = min(max(f, 0), n_in - 1)
        hi = min(max(f + 1, 0), n_in - 1)
        U[i, lo] += 1.0 - w
        U[i, hi] += w
    return U.astype(np.float32)


def _build_program(stage: int = 99):
    import concourse.bass as bass
    import concourse.bacc as bacc
    import concourse.tile as tile
    from concourse import mybir
    from contextlib import ExitStack

    f32 = mybir.dt.float32
    f16 = mybir.dt.float16
    AL = mybir.AluOpType

    nc = bacc.Bacc("TRN2", target_bir_lowering=False, debug=False,
                   num_devices=N_CORES)

    preds_d = nc.dram_tensor("preds", [K, HF, WF], f32, kind="ExternalInput")
    oh_d = nc.dram_tensor("oh", [4, 128, KQ], f16, kind="ExternalInput")
    ft_d = nc.dram_tensor("ft", [PIX // 128, 128, FTW], f16,
                          kind="ExternalInput")
    u16_d = nc.dram_tensor("u16", [HF, HC], f16, kind="ExternalInput")
    ident_d = nc.dram_tensor("ident", [64, 64], f16, kind="ExternalInput")
    out_d = nc.dram_tensor("out", [K2, FTW], f32, kind="ExternalOutput")

    HB = [(0, 0), (1, 0), (0, 1), (1, 1)]   # (half, wf-block), block-major

    with tile.TileContext(nc) as tc, ExitStack() as ctx:
        const_pool = ctx.enter_context(tc.tile_pool(name="const", bufs=1))
        joint_pool = ctx.enter_context(tc.tile_pool(name="joint", bufs=1))
        ft_pool = ctx.enter_context(tc.tile_pool(name="ft", bufs=1))
        res_pool = ctx.enter_context(tc.tile_pool(name="res", bufs=1))
        stg_pool = ctx.enter_context(tc.tile_pool(name="stg", bufs=1))
        tr_pool = ctx.enter_context(tc.tile_pool(name="trans", bufs=1))
        mx_pool = ctx.enter_context(tc.tile_pool(name="mx", bufs=2))
        ps1_pool = ctx.enter_context(
            tc.tile_pool(name="ps1", bufs=2, space="PSUM"))
        ps2_pool = ctx.enter_context(
            tc.tile_pool(name="ps2", bufs=1, space="PSUM"))
        pst_pool = ctx.enter_context(
            tc.tile_pool(name="pst", bufs=2, space="PSUM"))
        psf_pool = ctx.enter_context(
            tc.tile_pool(name="psf", bufs=1, space="PSUM"))

        # --- preds quarters: sole users of the sync DGE queue, issued
        # first, block-major so wf-block 0 completes at half time ---
        preds_t = {}
        for h, b in HB:
            pt = tr_pool.tile([128, KQ], f32, tag=f"preds{h}{b}")
            preds_t[(h, b)] = pt
            nc.sync.dma_start(
                pt[:].rearrange("p (k w) -> p k w", k=K),
                preds_d.ap()[:, h * 128:(h + 1) * 128,
                             b * 128:(b + 1) * 128].transpose([1, 0, 2]))

        # --- Pool DGE queue: one-hot quarters (host-built), u16, ident --
        oh_t = {}
        for i, (h, b) in enumerate(HB):
            oh = joint_pool.tile([128, KQ], f16, tag=f"oh{h}{b}")
            oh_t[(h, b)] = oh
            nc.gpsimd.dma_start(oh[:], oh_d.ap()[i, :, :])
        u16_t = []
        for h in range(2):
            t16 = const_pool.tile([128, HC], f16, tag=f"u16_{h}")
            nc.gpsimd.dma_start(t16[:], u16_d.ap()[h * 128:(h + 1) * 128, :])
            u16_t.append(t16)
        ident_t = const_pool.tile([64, 64], f16, tag="ident")
        nc.gpsimd.dma_start(ident_t[:], ident_d.ap()[:, :])

        # Pool warm-up: the first Q7 tensor op pays a ~13us ucode load;
        # spend it on a dummy multiply during the DMA lead-in.
        warm = const_pool.tile([128, 8], f16, tag="warm")
        nc.vector.memset(warm[:], 0.0)
        nc.gpsimd.tensor_tensor(warm[:], warm[:], warm[:], op=AL.mult)

        # stage tiles
        a_t = stg_pool.tile([64, K * WF], f16, tag="a")
        a3 = a_t[:].rearrange("p (k w) -> p k w", k=K)
        at_big = stg_pool.tile([128, 2 * HK], f16, tag="at")
        at4 = at_big[:].rearrange("p (w h k) -> p w h k", w=2, h=HC)
        b2 = stg_pool.tile([128, HK], f16, tag="b2")
        # zero the k=21 pad column of AT so stage 2 reads clean zeros
        nc.vector.memset(at4[:, :, :, K], 0.0)

        st2_ps = []
        if stage >= 3:
            for fc in range(0, HK, 512):
                ps = ps2_pool.tile([64, 512], f32, tag=f"st2_{fc}")
                st2_ps.append((fc, min(512, HK - fc), ps))

        # --- per quarter: DVE max tree + eq; joint-mul on Pool (last
        # quarter on DVE); per wf-block: stage1/transposes/stage2-half ---
        joint_t = {}
        for h, b in HB:
            p3 = preds_t[(h, b)][:].rearrange("p (k w) -> p k w", k=K)
            m1 = mx_pool.tile([128, 10 * 128], f32, tag="m1")
            m1v = m1[:].rearrange("p (k w) -> p k w", k=10)
            nc.vector.tensor_tensor(
                m1v, p3[:, 0:10, :], p3[:, 10:20, :], op=AL.max)
            nc.vector.tensor_tensor(
                m1v[:, 0:5, :], m1v[:, 0:5, :], m1v[:, 5:10, :], op=AL.max)
            nc.vector.tensor_tensor(
                m1v[:, 0:2, :], m1v[:, 0:2, :], m1v[:, 2:4, :], op=AL.max)
            nc.vector.tensor_tensor(
                m1v[:, 0:1, :], m1v[:, 0:1, :], m1v[:, 1:2, :], op=AL.max)
            nc.vector.tensor_tensor(
                m1v[:, 0:1, :], m1v[:, 0:1, :], m1v[:, 4:5, :], op=AL.max)
            mxv = mx_pool.tile([128, 128], f32, tag="mxv")
            nc.vector.tensor_tensor(
                mxv[:].unsqueeze(1), m1v[:, 0:1, :], p3[:, 20:21, :],
                op=AL.max)

            jq = joint_pool.tile([128, KQ], f16, tag=f"joint{h}{b}")
            joint_t[(h, b)] = jq
            nc.vector.tensor_tensor(
                jq[:].rearrange("p (k w) -> p k w", k=K), p3,
                mxv[:].unsqueeze(1).to_broadcast([128, K, 128]),
                op=AL.is_equal)
            meng = nc.vector if (h, b) == (1, 1) else nc.gpsimd
            meng.tensor_tensor(
                jq[:], jq[:], oh_t[(h, b)][:], op=AL.mult)

            if h != 1 or stage < 2:
                continue

            # ---- wf-block b complete in both halves: downsample it ----
            # stage 1: A[hc, kgroup, wf-block b] = Uh^T @ joint
            for g in range(6):
                k0, kw = 4 * g, min(4, K - 4 * g) * 128
                ps = ps1_pool.tile([64, 512], f32, tag="ps1")
                nc.tensor.matmul(
                    ps[:, :kw], u16_t[0][:, :],
                    joint_t[(0, b)][:, k0 * 128:k0 * 128 + kw],
                    start=True, stop=False)
                nc.tensor.matmul(
                    ps[:, :kw], u16_t[1][:, :],
                    joint_t[(1, b)][:, k0 * 128:k0 * 128 + kw],
                    start=False, stop=True)
                nc.scalar.copy(
                    a3[:, k0:k0 + kw // 128, b * 128:(b + 1) * 128],
                    ps[:, :kw].rearrange("p (k w) -> p k w", w=128))

            if stage < 3:
                continue
            # stage 1.5: transpose A's wf-half b, two classes per PSUM
            # tile; scatter copies on Scalar during the joint phase (b=0)
            # and on DVE after it ends (b=1)
            for k in range(0, K, 2):
                kn = min(2, K - k)
                pst = pst_pool.tile([128, 128], f16, tag="pst")
                for j in range(kn):
                    nc.tensor.transpose(
                        pst[:, j * 64:(j + 1) * 64],
                        a_t[:, (k + j) * WF + b * 128:
                            (k + j) * WF + b * 128 + 128],
                        ident_t[:])
                src = pst[:].rearrange("p (j h) -> p h j", j=2)[:, :, 0:kn]
                if b == 0:
                    nc.scalar.copy(at4[:, b, :, k:k + kn], src)
                else:
                    nc.vector.tensor_copy(at4[:, b, :, k:k + kn], src)

            # stage 2 half: accumulate Uw_b^T @ AT_b into held PSUM
            for fc, w, ps in st2_ps:
                nc.tensor.matmul(ps[:, :w], u16_t[b][:, :],
                                 at_big[:, b * HK + fc: b * HK + fc + w],
                                 start=(b == 0), stop=(b == 1))
                if b == 1:
                    nc.scalar.copy(b2[0:64, fc:fc + w], ps[:, :w])
                    if fc >= K2:
                        nc.vector.tensor_copy(
                            b2[64:128, fc - K2:fc + w - K2], ps[:, :w])
                    else:
                        nc.vector.tensor_copy(
                            b2[64:128, 0:w - K2], ps[:, K2:w])

        # feats^T load: trigger sits on the sync queue after the preds
        # quarters; the transfer overlaps the joint phase, data is needed
        # only by the final matmuls
        ft_big = ft_pool.tile([128, (PIX // 128) * FTW], f16, tag="ftbig")
        nc.sync.dma_start(
            ft_big[:].rearrange("p (x n) -> p x n", x=PIX // 128),
            ft_d.ap().transpose([1, 0, 2]))

        if stage <= 1:  # debug: dump joint slice for classes 0..K2-1
            dbg = res_pool.tile([128, K2], f32, tag="dbg")
            nc.scalar.copy(dbg[:], joint_t[(0, 0)][:, 0:K2])
            nc.sync.dma_start(out_d.ap()[:, 0:128].transpose([1, 0]), dbg[:])

        # ----- final: out[k, c] = sum_ch b2_ch^T @ ft_ch (+ counts) -----
        ftv = ft_big[:].rearrange("p (x n) -> p x n", x=PIX // 128)
        if stage >= 4:
            psum_o = psf_pool.tile([K2, FTW], f32, tag="fin")
            for ch in range(PIX // 128):
                nc.tensor.matmul(
                    psum_o[:, :],
                    b2[:, 2 * ch * K2: 2 * ch * K2 + K2],
                    ftv[:, ch, :],
                    start=(ch == 0), stop=(ch == PIX // 128 - 1))
            outc = res_pool.tile([K2, FTW], f32, tag="outc")
            nc.scalar.copy(outc[:], psum_o[:])
            nc.sync.dma_start(out_d.ap()[:, :], outc[:])

    nc.compile()
    return nc


def _get_program():
    if "nc" not in _PROGRAM_CACHE:
        _PROGRAM_CACHE["nc"] = _build_program()
    return _PROGRAM_CACHE["nc"]


def _host_inputs(feats, preds, masks):
    U = _upsample_matrix(HC, HF)
    u16 = U.astype(np.float16)
    ident = np.eye(64, dtype=np.float16)

    feats = np.asarray(feats, dtype=np.float32)
    preds = np.asarray(preds, dtype=np.float32)
    masks_i = np.asarray(masks).reshape(B, 2, 128, 2, 128)

    # one-hot of mask, quarter layout [(h,b) block-major, 128, (k, w)]
    ohq = np.empty((B, 4, 128, K, 128), dtype=np.float16)
    for i, (h, b) in enumerate([(0, 0), (1, 0), (0, 1), (1, 1)]):
        m = masks_i[:, h, :, b, :]                      # [B, 128, 128]
        ohq[:, i] = (m[:, :, None, :] ==
                     np.arange(K)[None, None, :, None])
    ohq = ohq.reshape(B, 4, 128, KQ)

    # feats^T [pix, c] fp16 with ones + zero-pad cols -> [32, 128, 258]
    ftp = np.empty((B, PIX, FTW), dtype=np.float16)
    ftp[:, :, :C] = feats.reshape(B, C, PIX).transpose(0, 2, 1)
    ftp[:, :, C] = 1.0
    ftp[:, :, C + 1] = 0.0

    in_maps = []
    for b in range(B):
        in_maps.append({
            "preds": np.ascontiguousarray(preds[b]),
            "oh": np.ascontiguousarray(ohq[b]),
            "ft": np.ascontiguousarray(ftp[b].reshape(PIX // 128, 128, FTW)),
            "u16": u16,
            "ident": ident,
        })
    return in_maps


def kernel(feats, preds, masks, _results_hook=None):
    from concourse.bass_utils import run_bass_kernel_spmd

    nc = _get_program()
    in_maps = _host_inputs(feats, preds, masks)
    res = run_bass_kernel_spmd(nc, in_maps, list(range(N_CORES)))
    if _results_hook is not None:
        _results_hook(res)

    protos = []
    for b in range(B):
        out = res.results[b]["out"]   # [K2, FTW] f32
        sums = out[:K, :C]            # [K, C]
        counts = out[:K, C]           # [K]
        protos.append(sums / (counts + EPS)[:, None])  # [K, C]
    return np.mean(np.stack(protos), axis=0).astype(np.float32)
